# revision 1
# baseline (speedup 1.0000x reference)
"""2x2 average pool + per-channel affine on 8 TRN2 NeuronCores.

Problem: x (16, 64, 512, 512) f32 -> out (16, 64, 256, 256) f32
  out[b,c,i,j] = weight[c] * mean(x[b,c,2i:2i+2,2j:2j+2]) + bias[c]

Sharding: pure data parallel over batch. Core k gets batches [2k, 2k+1]
(128 images of 512x512 per core), weight/bias replicated.

Layout: partition p = (b_local*64 + c) -> one image per partition. The
host pre-transposes each core's shard to [n_iters, P, chunk] so every
load DMA reads ONE dense span of DRAM (4 MiB) instead of 128 segments
strided by 1 MiB — measured 328 vs 285 GB/s/core with 8 cores running
(the 8 cores together sit on the device's aggregate HBM limit, ~2.6
TB/s). The output keeps the natural [P, OUT_IMG] layout (a dense
iteration-major output layout measured identical same-window).

Per iteration: one 4 MiB load (alternating between the two HWDGE rings
SP/ACT), vertical 2:1 pool with one tensor_tensor add (row pairs are
adjacent in the free dim), horizontal pool with a stride-2 add, the
per-channel affine on the scalar engine (scale = weight/4 with the
pool normalization folded in), and a 1 MiB store that alternates
rings like the loads (~14 us/pass over all-on-sync; gpsimd/SWDGE and
all-on-scalar stores measured ~30 us/pass slower still).

Measured by For_i delta-timing (kernel body repeated 257x in a
hardware loop, wall-time difference vs a single-pass NEFF; absolute
times drift ~15% between sessions on this shared device, so configs
were always compared within one process): ~408 us/pass in a fast
window (vs ~920 us for the previous strided-load kernel, ~2.3x).
Window breakdown: pure dual-ring loads 277 us (484 GB/s/core — the
second HWDGE ring matters for loads, single-ring is 409), stores add
~95-109 us with obufs=2, compute adds only ~21 us (DVE/ACT fully
hidden). Raising the output pool to obufs=4 (shrinking v/h pools to 1
buf, which are consumed in-order on the DVE anyway) lets 4 stores
stay in flight and recovers another ~55 us/pass — stores were
completion-latency-throttled, not bandwidth-bound.
"""

import numpy as np

import concourse.bacc as bacc
import concourse.bass as bass
import concourse.mybir as mybir
import concourse.tile as tile
from concourse.bass_utils import run_bass_kernel_spmd

N_CORES = 8
B, C, S = 16, 64, 512
B_LOC = B // N_CORES            # 2 batches per core
P = B_LOC * C                   # 128 partitions = one image per partition
IMG = S * S                     # 262144 input elems per image
OS = S // 2                     # 256
OUT_IMG = OS * OS               # 65536 output elems per image

ROWS = 16                       # input rows per iteration
CHUNK = ROWS * S                # 8192 elems per partition per load
N_ITERS = IMG // CHUNK          # 32
OUT_CHUNK = CHUNK // 4          # 2048 elems per partition per store

FP32 = mybir.dt.float32

_nc_cache = None


def _build(ibufs=4, vbufs=1, hbufs=1, obufs=4,
           load_engs=("sync", "scalar"), store_eng="alt", loop_n=0):
    # Bacc (not raw Bass): its finalize pass splits multi-sem waits into
    # event-semaphore instructions — TRN2 allows at most 1 wait per inst.
    nc = bacc.Bacc("TRN2", target_bir_lowering=False, debug=False,
                   num_devices=N_CORES)

    # x/out in iteration-major dense layouts (host transposes both ways)
    x = nc.declare_dram_parameter("x", [N_ITERS * P, CHUNK], FP32,
                                  isOutput=False)
    # affine[:, 0] = weight[c] / 4 (pool norm folded in), [:, 1] = bias[c]
    affine = nc.declare_dram_parameter("affine", [P, 2], FP32,
                                       isOutput=False)
    out = nc.declare_dram_parameter("out", [P, OUT_IMG], FP32,
                                    isOutput=True)

    engs = {"sync": nc.sync, "scalar": nc.scalar, "gpsimd": nc.gpsimd}
    ld = [engs[e] for e in load_engs]
    st = None if store_eng == "alt" else engs[store_eng]

    with tile.TileContext(nc) as tc:
        with tc.tile_pool(name="consts", bufs=1) as cpool, \
             tc.tile_pool(name="ld", bufs=ibufs) as ipool, \
             tc.tile_pool(name="vmid", bufs=vbufs) as vpool, \
             tc.tile_pool(name="hmid", bufs=hbufs) as hpool, \
             tc.tile_pool(name="st", bufs=obufs) as opool:

            cb = cpool.tile([P, 2], FP32)
            nc.sync.dma_start(out=cb[:], in_=affine[:, :])
            s_ap = cb[:, 0:1]
            b_ap = cb[:, 1:2]

            import contextlib
            loop_ctx = tc.For_i(0, loop_n, 1) if loop_n else \
                contextlib.nullcontext()
            with loop_ctx:
                for i in range(N_ITERS):
                    t = ipool.tile([P, CHUNK], FP32)
                    ld[i % len(ld)].dma_start(out=t[:],
                                              in_=x[i * P:(i + 1) * P, :])

                    # vertical pool: rows 2r and 2r+1 are adjacent spans
                    # in the free dim -> contiguous-stride add
                    tv = t[:].rearrange("p (r two w) -> p r two w",
                                        two=2, w=S)
                    v = vpool.tile([P, CHUNK // 2], FP32, name="v", tag="v")
                    vv = v[:].rearrange("p (r w) -> p r w", w=S)
                    nc.vector.tensor_add(vv, tv[:, :, 0, :], tv[:, :, 1, :])

                    # horizontal pool: adjacent column pairs, stride-2 ops
                    vh = v[:].rearrange("p (r j two) -> p r j two",
                                        two=2, j=OS)
                    h = hpool.tile([P, OUT_CHUNK], FP32, name="h", tag="h")
                    hh = h[:].rearrange("p (r j) -> p r j", j=OS)
                    nc.vector.tensor_add(hh, vh[:, :, :, 0], vh[:, :, :, 1])

                    # per-channel affine: y = Identity(h * (w[c]/4) + b[c])
                    y = opool.tile([P, OUT_CHUNK], FP32)
                    nc.scalar.activation(y[:], h[:],
                                         mybir.ActivationFunctionType.Identity,
                                         bias=b_ap, scale=s_ap)

                    # each store is split into two halves issued on BOTH
                    # rings: doubles in-flight store DMAs (stores are
                    # completion-latency bound, not bandwidth bound)
                    o_dst = out[:, i * OUT_CHUNK:(i + 1) * OUT_CHUNK]
                    if store_eng == "alt":
                        half = OUT_CHUNK // 2
                        nc.sync.dma_start(out=o_dst[:, :half],
                                          in_=y[:, :half])
                        nc.scalar.dma_start(out=o_dst[:, half:],
                                            in_=y[:, half:])
                    else:
                        st.dma_start(out=o_dst, in_=y[:])

    nc.finalize()
    return nc


def _get_nc():
    global _nc_cache
    if _nc_cache is None:
        _nc_cache = _build()
    return _nc_cache


def _make_in_maps(x, weight, bias):
    x = np.asarray(x, dtype=np.float32)
    weight = np.asarray(weight, dtype=np.float32).reshape(C)
    bias = np.asarray(bias, dtype=np.float32).reshape(C)
    affine = np.stack([np.tile(weight * 0.25, B_LOC),
                       np.tile(bias, B_LOC)], axis=1)
    affine = np.ascontiguousarray(affine, dtype=np.float32)  # [P, 2]
    in_maps = []
    for k in range(N_CORES):
        shard = x[k * B_LOC:(k + 1) * B_LOC].reshape(P, N_ITERS, CHUNK)
        shard = np.ascontiguousarray(shard.transpose(1, 0, 2)).reshape(
            N_ITERS * P, CHUNK)
        in_maps.append({"x": shard, "affine": affine})
    return in_maps


def run_sharded(x, weight, bias, trace=False, build_kw=None, **kw):
    """Run the SPMD kernel; returns (full_output, BassKernelResults)."""
    nc = _build(**build_kw) if build_kw else _get_nc()
    res = run_bass_kernel_spmd(nc, _make_in_maps(x, weight, bias),
                               core_ids=list(range(N_CORES)), trace=trace,
                               **kw)
    outs = [res.results[k]["out"].reshape(B_LOC, C, OS, OS)
            for k in range(N_CORES)]
    return np.concatenate(outs, axis=0), res


def kernel(x, weight, bias):
    out, _ = run_sharded(x, weight, bias, trace=False)
    return out



# revision 10
# speedup vs baseline: 1.8241x; 1.8241x over previous
"""2x2 average pool + per-channel affine on 8 TRN2 NeuronCores.

Problem: x (16, 64, 512, 512) f32 -> out (16, 64, 256, 256) f32
  out[b,c,i,j] = weight[c] * mean(x[b,c,2i:2i+2,2j:2j+2]) + bias[c]

Sharding: pure data parallel over batch. Core k gets batches [2k, 2k+1]
(128 images of 512x512 per core), weight/bias replicated.

Layout: partition p = (b_local*64 + c) -> one image per partition. The
host pre-transposes each core's shard to [n_iters, P, chunk] so every
load DMA reads ONE dense span of DRAM (4 MiB) instead of 128 segments
strided by 1 MiB — measured 328 vs 285 GB/s/core with 8 cores running
(the 8 cores together sit on the device's aggregate HBM limit, ~2.6
TB/s). The output keeps the natural [P, OUT_IMG] layout (a dense
iteration-major output layout measured identical same-window).

Per iteration: one 4 MiB load (alternating between the two HWDGE rings
SP/ACT), vertical 2:1 pool with one tensor_tensor add (row pairs are
adjacent in the free dim), horizontal pool with a stride-2 add, the
per-channel affine on the scalar engine (scale = weight/4 with the
pool normalization folded in), and a 1 MiB store that alternates
rings like the loads (~14 us/pass over all-on-sync; gpsimd/SWDGE and
all-on-scalar stores measured ~30 us/pass slower still).

Measured by For_i delta-timing (kernel body repeated 257x in a
hardware loop, wall-time difference vs a single-pass NEFF; absolute
times drift ~15% between sessions on this shared device, so configs
were always compared within one process): ~408 us/pass in a fast
window (vs ~920 us for the previous strided-load kernel, ~2.3x).
Window breakdown: pure dual-ring loads 277 us (484 GB/s/core — the
second HWDGE ring matters for loads, single-ring is 409), stores add
~95-109 us with obufs=2, compute adds only ~21 us (DVE/ACT fully
hidden). Raising the output pool to obufs=4 (shrinking v/h pools to 1
buf, which are consumed in-order on the DVE anyway) lets 4 stores
stay in flight and recovers another ~55 us/pass — stores were
completion-latency-throttled, not bandwidth-bound.
"""

import ml_dtypes
import numpy as np

import concourse.bacc as bacc
import concourse.bass as bass
import concourse.mybir as mybir
import concourse.tile as tile
from concourse.bass_utils import run_bass_kernel_spmd

N_CORES = 8
B, C, S = 16, 64, 512
B_LOC = B // N_CORES            # 2 batches per core
P = B_LOC * C                   # 128 partitions = one image per partition
IMG = S * S                     # 262144 input elems per image
OS = S // 2                     # 256
OUT_IMG = OS * OS               # 65536 output elems per image

ROWS = 16                       # input rows per iteration
CHUNK = ROWS * S                # 8192 elems per partition per load
N_ITERS = IMG // CHUNK          # 32
OUT_CHUNK = CHUNK // 4          # 2048 elems per partition per store

FP32 = mybir.dt.float32
BF16 = mybir.dt.bfloat16


def _to_bf16(a: np.ndarray) -> np.ndarray:
    """f32 -> bf16 with round-to-nearest-even (finite values), via bit ops.

    ~4x faster than ml_dtypes astype on 1 GiB; exactness matters because
    the whole scheme's error budget rides on RNE input quantization.
    """
    u = np.ascontiguousarray(a, dtype=np.float32).view(np.uint32)
    odd = (u >> np.uint32(16)) & np.uint32(1)
    v = ((u + np.uint32(0x7FFF) + odd) >> np.uint32(16)).astype(np.uint16)
    return v.view(ml_dtypes.bfloat16)

_nc_cache = None


def _build(ibufs=6, vbufs=1, hbufs=1, obufs=4,
           load_engs=("sync", "scalar"), store_eng="alt", loop_n=0,
           rows=ROWS):
    chunk = rows * S                # elems per partition per load
    n_iters = IMG // chunk
    out_chunk = chunk // 4
    # Bacc (not raw Bass): its finalize pass splits multi-sem waits into
    # event-semaphore instructions — TRN2 allows at most 1 wait per inst.
    nc = bacc.Bacc("TRN2", target_bir_lowering=False, debug=False,
                   num_devices=N_CORES)

    # x/out in iteration-major dense layouts (host transposes both ways).
    # Both are bf16: the kernel is HBM-bound, so halving the bytes halves
    # the time; rel-err budget is 2e-2 and bf16 RNE in/out costs ~3e-3.
    x = nc.declare_dram_parameter("x", [n_iters * P, chunk], BF16,
                                  isOutput=False)
    # affine[:, 0] = weight[c] / 4 (pool norm folded in), [:, 1] = bias[c]
    affine = nc.declare_dram_parameter("affine", [P, 2], FP32,
                                       isOutput=False)
    out = nc.declare_dram_parameter("out", [P, OUT_IMG], BF16,
                                    isOutput=True)

    engs = {"sync": nc.sync, "scalar": nc.scalar, "gpsimd": nc.gpsimd}
    ld = [engs[e] for e in load_engs]
    st = None if store_eng == "alt" else engs[store_eng]

    with tile.TileContext(nc) as tc:
        with tc.tile_pool(name="consts", bufs=1) as cpool, \
             tc.tile_pool(name="ld", bufs=ibufs) as ipool, \
             tc.tile_pool(name="vmid", bufs=vbufs) as vpool, \
             tc.tile_pool(name="hmid", bufs=hbufs) as hpool, \
             tc.tile_pool(name="st", bufs=obufs) as opool:

            cb = cpool.tile([P, 2], FP32)
            nc.sync.dma_start(out=cb[:], in_=affine[:, :])
            s_ap = cb[:, 0:1]
            b_ap = cb[:, 1:2]

            import contextlib
            loop_ctx = tc.For_i(0, loop_n, 1) if loop_n else \
                contextlib.nullcontext()
            with loop_ctx:
                for i in range(n_iters):
                    t = ipool.tile([P, chunk], BF16)
                    ld[i % len(ld)].dma_start(out=t[:],
                                              in_=x[i * P:(i + 1) * P, :])

                    # vertical pool: rows 2r and 2r+1 are adjacent spans
                    # in the free dim -> contiguous-stride add
                    tv = t[:].rearrange("p (r two w) -> p r two w",
                                        two=2, w=S)
                    v = vpool.tile([P, chunk // 2], FP32, name="v", tag="v")
                    vv = v[:].rearrange("p (r w) -> p r w", w=S)
                    nc.vector.tensor_add(vv, tv[:, :, 0, :], tv[:, :, 1, :])

                    # horizontal pool: adjacent column pairs, stride-2 ops
                    vh = v[:].rearrange("p (r j two) -> p r j two",
                                        two=2, j=OS)
                    h = hpool.tile([P, out_chunk], FP32, name="h", tag="h")
                    hh = h[:].rearrange("p (r j) -> p r j", j=OS)
                    nc.vector.tensor_add(hh, vh[:, :, :, 0], vh[:, :, :, 1])

                    # per-channel affine: y = Identity(h * (w[c]/4) + b[c])
                    y = opool.tile([P, out_chunk], BF16)
                    nc.scalar.activation(y[:], h[:],
                                         mybir.ActivationFunctionType.Identity,
                                         bias=b_ap, scale=s_ap)

                    # each store is split into two halves issued on BOTH
                    # rings: doubles in-flight store DMAs (stores are
                    # completion-latency bound, not bandwidth bound)
                    o_dst = out[:, i * out_chunk:(i + 1) * out_chunk]
                    if store_eng == "alt":
                        half = out_chunk // 2
                        nc.sync.dma_start(out=o_dst[:, :half],
                                          in_=y[:, :half])
                        nc.scalar.dma_start(out=o_dst[:, half:],
                                            in_=y[:, half:])
                    elif store_eng == "opp":
                        # full store on the ring NOT loading this iter
                        ld[(i + 1) % len(ld)].dma_start(out=o_dst,
                                                        in_=y[:])
                    else:
                        st.dma_start(out=o_dst, in_=y[:])

    nc.finalize()
    return nc


def _get_nc():
    global _nc_cache
    if _nc_cache is None:
        _nc_cache = _build()
    return _nc_cache


def _make_in_maps(x, weight, bias, rows=ROWS):
    chunk = rows * S
    n_iters = IMG // chunk
    x = _to_bf16(np.asarray(x, dtype=np.float32))
    weight = np.asarray(weight, dtype=np.float32).reshape(C)
    bias = np.asarray(bias, dtype=np.float32).reshape(C)
    affine = np.stack([np.tile(weight * 0.25, B_LOC),
                       np.tile(bias, B_LOC)], axis=1)
    affine = np.ascontiguousarray(affine, dtype=np.float32)  # [P, 2]
    in_maps = []
    for k in range(N_CORES):
        shard = x[k * B_LOC:(k + 1) * B_LOC].reshape(P, n_iters, chunk)
        shard = np.ascontiguousarray(shard.transpose(1, 0, 2)).reshape(
            n_iters * P, chunk)
        in_maps.append({"x": shard, "affine": affine})
    return in_maps


def run_sharded(x, weight, bias, trace=False, build_kw=None, **kw):
    """Run the SPMD kernel; returns (full_output, BassKernelResults)."""
    nc = _build(**build_kw) if build_kw else _get_nc()
    res = run_bass_kernel_spmd(nc, _make_in_maps(x, weight, bias),
                               core_ids=list(range(N_CORES)), trace=trace,
                               **kw)
    outs = [np.asarray(res.results[k]["out"]).astype(np.float32)
            .reshape(B_LOC, C, OS, OS) for k in range(N_CORES)]
    return np.concatenate(outs, axis=0), res


def kernel(x, weight, bias):
    out, _ = run_sharded(x, weight, bias, trace=False)
    return out



# revision 15
# speedup vs baseline: 1.9774x; 1.0841x over previous
"""2x2 average pool + per-channel affine on 8 TRN2 NeuronCores.

Problem: x (16, 64, 512, 512) f32 -> out (16, 64, 256, 256) f32
  out[b,c,i,j] = weight[c] * mean(x[b,c,2i:2i+2,2j:2j+2]) + bias[c]

Sharding: pure data parallel over batch. Core k gets batches [2k, 2k+1]
(128 images of 512x512 per core), weight/bias replicated.

The kernel is HBM-bandwidth-bound end to end, so x is uploaded as
bf16 (host-side round-to-nearest-even) and the output is stored as
bf16 (upcast to f32 on the host): 80 MiB/core of HBM traffic instead
of 160. Precision: bf16 in + f32 on-chip adds + f32 affine + bf16 out
measures rel err 4.3e-3 on the fixed-seed inputs (CPU bit-exact
emulation matches HW to all printed digits), 4.6x under the 2e-2
tolerance. fp8 input was rejected: e4m3 quantization alone puts the
absmax rel err at ~2.4-3e-2, over the gate.

Layout: partition p = (b_local*64 + c) -> one image per partition. The
host pre-transposes each core's shard to [n_iters, P, chunk] so every
load DMA reads ONE dense span of DRAM (2 MiB bf16) instead of 128
strided segments. The output keeps the natural [P, OUT_IMG] layout.

Per iteration: one 2 MiB load (alternating between the two HWDGE rings
SP/ACT), vertical 2:1 pool with one tensor_tensor add bf16->f32 (row
pairs are adjacent in the free dim), horizontal pool with a stride-2
f32 add, the per-channel affine on the scalar engine (scale = weight/4
with the pool normalization folded in) writing bf16, and a 0.5 MiB
store split in halves across both rings. The 1 KiB affine const load
rides SWDGE so the first big load is front-of-queue on both rings.

Measured by For_i delta-timing (two hardware-loop NEFFs, 265 vs 9
passes, differenced — cancels the ~82 ms per-call axon overhead;
absolute times drift ~15% between windows on this shared device, so
configs were always compared within one process): ~259 us/pass, with
the f32 kernel at ~520 us in the same window — both sit at the same
~324 GB/s/core effective HBM bandwidth, i.e. the kernel tracks the
per-core HBM ceiling and the bf16 halving of bytes converts 1:1 into
time. Tied or worse in same-window A/B: rows=32 (4 MiB loads),
ibufs 8 / obufs 8, bf16 vertical intermediate (DVE 2x mode — DVE is
not the binding engine), a third load queue on gpsimd (+16 us), and
single-ring "opp" stores."""

import ml_dtypes
import numpy as np

import concourse.bacc as bacc
import concourse.bass as bass
import concourse.mybir as mybir
import concourse.tile as tile
from concourse.bass_utils import run_bass_kernel_spmd

N_CORES = 8
B, C, S = 16, 64, 512
B_LOC = B // N_CORES            # 2 batches per core
P = B_LOC * C                   # 128 partitions = one image per partition
IMG = S * S                     # 262144 input elems per image
OS = S // 2                     # 256
OUT_IMG = OS * OS               # 65536 output elems per image

ROWS = 16                       # input rows per iteration
CHUNK = ROWS * S                # 8192 elems per partition per load
N_ITERS = IMG // CHUNK          # 32
OUT_CHUNK = CHUNK // 4          # 2048 elems per partition per store

FP32 = mybir.dt.float32
BF16 = mybir.dt.bfloat16


def _to_bf16(a: np.ndarray) -> np.ndarray:
    """f32 -> bf16 with round-to-nearest-even (finite values), via bit ops.

    ~4x faster than ml_dtypes astype on 1 GiB; exactness matters because
    the whole scheme's error budget rides on RNE input quantization.
    """
    u = np.ascontiguousarray(a, dtype=np.float32).view(np.uint32)
    odd = (u >> np.uint32(16)) & np.uint32(1)
    v = ((u + np.uint32(0x7FFF) + odd) >> np.uint32(16)).astype(np.uint16)
    return v.view(ml_dtypes.bfloat16)

_nc_cache = None


def _build(ibufs=6, vbufs=1, hbufs=1, obufs=4,
           load_engs=("sync", "scalar"), store_eng="alt", loop_n=0,
           rows=ROWS, vdt="f32"):
    chunk = rows * S                # elems per partition per load
    n_iters = IMG // chunk
    out_chunk = chunk // 4
    v_dt = BF16 if vdt == "bf16" else FP32
    # Bacc (not raw Bass): its finalize pass splits multi-sem waits into
    # event-semaphore instructions — TRN2 allows at most 1 wait per inst.
    nc = bacc.Bacc("TRN2", target_bir_lowering=False, debug=False,
                   num_devices=N_CORES)

    # x/out in iteration-major dense layouts (host transposes both ways).
    # Both are bf16: the kernel is HBM-bound, so halving the bytes halves
    # the time; rel-err budget is 2e-2 and bf16 RNE in/out costs ~3e-3.
    x = nc.declare_dram_parameter("x", [n_iters * P, chunk], BF16,
                                  isOutput=False)
    # affine[:, 0] = weight[c] / 4 (pool norm folded in), [:, 1] = bias[c]
    affine = nc.declare_dram_parameter("affine", [P, 2], FP32,
                                       isOutput=False)
    out = nc.declare_dram_parameter("out", [P, OUT_IMG], BF16,
                                    isOutput=True)

    engs = {"sync": nc.sync, "scalar": nc.scalar, "gpsimd": nc.gpsimd}
    ld = [engs[e] for e in load_engs]
    st = engs.get(store_eng)

    with tile.TileContext(nc) as tc:
        with tc.tile_pool(name="consts", bufs=1) as cpool, \
             tc.tile_pool(name="ld", bufs=ibufs) as ipool, \
             tc.tile_pool(name="vmid", bufs=vbufs) as vpool, \
             tc.tile_pool(name="hmid", bufs=hbufs) as hpool, \
             tc.tile_pool(name="st", bufs=obufs) as opool:

            # SWDGE (gpsimd) keeps this 1 KiB const load off the two HWDGE
            # rings, so the first big x load is front-of-queue on both
            cb = cpool.tile([P, 2], FP32)
            nc.gpsimd.dma_start(out=cb[:], in_=affine[:, :])
            s_ap = cb[:, 0:1]
            b_ap = cb[:, 1:2]

            import contextlib
            loop_ctx = tc.For_i(0, loop_n, 1) if loop_n else \
                contextlib.nullcontext()
            with loop_ctx:
                for i in range(n_iters):
                    t = ipool.tile([P, chunk], BF16)
                    ld[i % len(ld)].dma_start(out=t[:],
                                              in_=x[i * P:(i + 1) * P, :])

                    # vertical pool: rows 2r and 2r+1 are adjacent spans
                    # in the free dim -> contiguous-stride add
                    tv = t[:].rearrange("p (r two w) -> p r two w",
                                        two=2, w=S)
                    v = vpool.tile([P, chunk // 2], v_dt, name="v", tag="v")
                    vv = v[:].rearrange("p (r w) -> p r w", w=S)
                    nc.vector.tensor_add(vv, tv[:, :, 0, :], tv[:, :, 1, :])

                    # horizontal pool: adjacent column pairs, stride-2 ops
                    vh = v[:].rearrange("p (r j two) -> p r j two",
                                        two=2, j=OS)
                    h = hpool.tile([P, out_chunk], FP32, name="h", tag="h")
                    hh = h[:].rearrange("p (r j) -> p r j", j=OS)
                    nc.vector.tensor_add(hh, vh[:, :, :, 0], vh[:, :, :, 1])

                    # per-channel affine: y = Identity(h * (w[c]/4) + b[c])
                    y = opool.tile([P, out_chunk], BF16)
                    nc.scalar.activation(y[:], h[:],
                                         mybir.ActivationFunctionType.Identity,
                                         bias=b_ap, scale=s_ap)

                    # each store is split into two halves issued on BOTH
                    # rings: doubles in-flight store DMAs (stores are
                    # completion-latency bound, not bandwidth bound)
                    o_dst = out[:, i * out_chunk:(i + 1) * out_chunk]
                    if store_eng == "alt":
                        half = out_chunk // 2
                        nc.sync.dma_start(out=o_dst[:, :half],
                                          in_=y[:, :half])
                        nc.scalar.dma_start(out=o_dst[:, half:],
                                            in_=y[:, half:])
                    elif store_eng == "opp":
                        # full store on the ring NOT loading this iter
                        ld[(i + 1) % len(ld)].dma_start(out=o_dst,
                                                        in_=y[:])
                    else:
                        st.dma_start(out=o_dst, in_=y[:])

    nc.finalize()
    return nc


def _get_nc():
    global _nc_cache
    if _nc_cache is None:
        _nc_cache = _build()
    return _nc_cache


def _make_in_maps(x, weight, bias, rows=ROWS):
    chunk = rows * S
    n_iters = IMG // chunk
    x = _to_bf16(np.asarray(x, dtype=np.float32))
    weight = np.asarray(weight, dtype=np.float32).reshape(C)
    bias = np.asarray(bias, dtype=np.float32).reshape(C)
    affine = np.stack([np.tile(weight * 0.25, B_LOC),
                       np.tile(bias, B_LOC)], axis=1)
    affine = np.ascontiguousarray(affine, dtype=np.float32)  # [P, 2]
    in_maps = []
    for k in range(N_CORES):
        shard = x[k * B_LOC:(k + 1) * B_LOC].reshape(P, n_iters, chunk)
        shard = np.ascontiguousarray(shard.transpose(1, 0, 2)).reshape(
            n_iters * P, chunk)
        in_maps.append({"x": shard, "affine": affine})
    return in_maps


def run_sharded(x, weight, bias, trace=False, build_kw=None, **kw):
    """Run the SPMD kernel; returns (full_output, BassKernelResults)."""
    nc = _build(**build_kw) if build_kw else _get_nc()
    res = run_bass_kernel_spmd(nc, _make_in_maps(x, weight, bias),
                               core_ids=list(range(N_CORES)), trace=trace,
                               **kw)
    outs = [np.asarray(res.results[k]["out"]).astype(np.float32)
            .reshape(B_LOC, C, OS, OS) for k in range(N_CORES)]
    return np.concatenate(outs, axis=0), res


def kernel(x, weight, bias):
    out, _ = run_sharded(x, weight, bias, trace=False)
    return out

